# revision 26
# baseline (speedup 1.0000x reference)
"""Trainium2 Bass kernel for ExampleGuidedAttention (N=8, C=256, H=W=64).

Data-parallel over batch N across 8 NeuronCores; each core computes one
batch element's full guided attention.

Algorithm notes (per core):
  q = conv_w @ src_pix                      [64, 4096]   (PE, bf16)
  S^T[j,i] = sum_o q[o,j] q[o,i]            (PE, bf16; S symmetric; two
             j-blocks packed in the 128x128 array via tile_position)
  F[j,i] = exp(S^T[j,i] - 64 + 13*ln2)      (ACT; global shift keeps the
             fp32 exp in range; the 2^13 factor cancels against 1/Z)
  Per column-slice s (512 pixels) the F tiles span ALL j, so
  Z[i] = sum_j F[j,i] is computed per-slice with ones-vector matmuls on
  the PE, and each slice normalizes + blends + stores immediately --
  no full-image unnormalized buffer and no end-of-kernel fixup tail.

  Off-diagonal j-block tiles (28 of 32 per slice) are stored in fp8-e4m3
  and applied with DoubleRow matmuls (2 j-blocks per pass, 2x PE rate).
  The 4 diagonal-crossing tiles stay bf16 so the dominant near-diagonal
  attention terms keep full precision; all tiles share the 2^13 scale so
  they accumulate consistently in PSUM and the scale cancels in 1/Z.

  out = [ (1-m)*ref_att*invZ + m*ref ; src_att*invZ ]

The issue order software-pipelines slice s's scores/exp chain (ACT
bound, single PSUM group buffer) against slice s-1's apply matmuls so
the PE never stalls on the exp chain.
"""

import math

import numpy as np

import concourse.bass as bass
import concourse.mybir as mybir
import concourse.tile as tile
from concourse import bacc, bass_utils
from concourse.bass import ts
from concourse.alu_op_type import AluOpType

P = 128
C = 256          # feature channels
CQ = 64          # query channels
HW = 4096        # pixels per image
NB = HW // P     # 32 pixel blocks (contraction chunks)
SLICE = 512
NS = HW // SLICE  # 8 output column slices
NG = NB // 4      # 8 score groups of 4 j-blocks per slice
NCORES = 8
K_SCALE = 13.0    # F scaled by 2^13: off-diag fp8 overflow headroom to S=70

F32 = mybir.dt.float32
BF16 = mybir.dt.bfloat16
F8 = mybir.dt.float8e4
EXP = mybir.ActivationFunctionType.Exp
DR = mybir.MatmulPerfMode.DoubleRow


def _build_body(tc, src, ref, mask, wT, out):
    nc = tc.nc
    src_r = src.ap().rearrange("(ci p) j -> p ci j", p=P)   # [128, 2, 4096]
    ref_r = ref.ap().rearrange("(ci p) j -> p ci j", p=P)
    wT_r = wT.ap().rearrange("(ci p) o -> p ci o", p=P)     # [128, 2, 128]
    out_r = out.ap().rearrange("(cb p) j -> cb p j", p=P)   # [4, 128, 4096]

    with (
        tc.tile_pool(name="persist", bufs=1) as persist,
        tc.tile_pool(name="ps_sc", bufs=1, space="PSUM") as ps_sc,
        tc.tile_pool(name="ps_ap", bufs=2, space="PSUM") as ps_ap,
        tc.tile_pool(name="ps_z", bufs=1, space="PSUM") as ps_z,
        tc.tile_pool(name="dram", bufs=1, space="DRAM") as dram,
    ):
        refb_h = [
            persist.tile([P, 2, HW // 2], BF16, name=f"refb{h}")
            for h in range(2)
        ]
        q2 = persist.tile([P, HW], BF16)
        pixT_bf = persist.tile([P, NB, 2 * C], BF16)   # [src 256 | ref 256]
        pixT_f8 = persist.tile([P, NB, 2 * C], F8)
        wT_sb = persist.tile([P, 2, 2 * CQ], BF16)
        mask_rep = persist.tile([P, HW], BF16)
        omask_rep = persist.tile([P, HW], BF16)        # 1 - mask
        mref = persist.tile([P, 2, HW], BF16)          # mask * ref
        exp_bias = persist.tile([P, 1], F32)
        ones8 = persist.tile([P, 2, 16], F8)  # 16B k-pair stride for dual-fp8 ldweights
        ones_bf = persist.tile([P, 2], BF16)
        warm_sb = persist.tile([P, SLICE], BF16)
        zrow = dram.tile([NS, SLICE], F32)
        nc.vector.memset(exp_bias, -64.0 + K_SCALE * math.log(2.0))
        nc.vector.memset(ones8, 1.0)
        nc.vector.memset(ones_bf, 1.0)
        nc.vector.memset(warm_sb, 0.0)

        nc.sync.dma_start(out=wT_sb, in_=wT_r)
        for s in range(NS):
            nc.gpsimd.dma_start(
                out=mask_rep[:, ts(s, SLICE)],
                in_=mask.ap()[ts(s, SLICE)].partition_broadcast(P),
            )

        with tc.tile_pool(name="early", bufs=6) as early, \
             tc.tile_pool(name="early1", bufs=1) as early1:
            # PE warmup: back-to-back matmuls on zeroed data keep the HAM
            # clock gate at 8/8 (2.4 GHz) while input DMAs stream in.
            warm_ps = ps_sc.tile([P, 4, SLICE], F32, name="pss", tag="pss")
            for r in range(30):
                nc.tensor.matmul(
                    warm_ps[:, r % 4, :], warm_sb[:, 0:P], warm_sb,
                    start=True, stop=True,
                )
            # warm the ACT exp table so slice 0 doesn't pay the table load
            nc.scalar.activation(
                out=warm_sb[:, 0:1], in_=warm_sb[:, 0:1], func=EXP,
                bias=exp_bias,
            )

            srcb_h = [
                early1.tile([P, 2, HW // 2], BF16, name=f"srcb{h}")
                for h in range(2)
            ]
            # fp32 inputs via fast hardware DMA.  src casts on DVE (conv ->
            # q gates everything); ref casts on ACT so the DVE can proceed
            # to q2 / fp8 work without head-of-line blocking.
            # src in 8 0.5MB chunks: each dma_start lands on its own
            # hardware queue (~85GB/s each).  ref is issued on sync AFTER
            # the src transposes so src gets the full HBM read bandwidth
            # first (conv -> q -> scores gates everything; ref is needed
            # ~30us later).
            src_stages, ref_stages = [], []
            SC = SLICE // 2
            for c in range(16):
                st = early.tile([P, 2, SC], F32, name="stage", tag="st")
                (nc.sync if c % 2 == 0 else nc.scalar).dma_start(
                    out=st, in_=src_r[:, :, ts(c, SC)]
                )
                src_stages.append(st)
            for c in range(16):
                nc.vector.tensor_copy(
                    out=srcb_h[c // 8][:, :, ts(c % 8, SC)],
                    in_=src_stages[c],
                )
            # 1x1 conv: q = wT.T @ src_pix; q into both partition halves.
            # 3 warm matmuls between slices keep the PE (and HAM clock)
            # busy while the next slice's DMA+cast lands.
            for s in range(NS):
                sl = ts(s, SLICE)
                psq = ps_z.tile([P, SLICE], F32, name="psz", tag="psz")
                for ci in range(2):
                    nc.tensor.matmul(
                        psq,
                        wT_sb[:, ci, :],
                        srcb_h[s // 4][:, ci, ts(s % 4, SLICE)],
                        start=(ci == 0),
                        stop=(ci == 1),
                    )
                for r in range(3):
                    nc.tensor.matmul(
                        warm_ps[:, r, :], warm_sb[:, 0:P], warm_sb,
                        start=True, stop=True,
                    )
                nc.vector.tensor_copy(out=q2[:, sl], in_=psq)
            # XBAR transposes: pixT[p, b, c] = pix[c, b*128+p]; src halves
            # first (the j-half split lets each start as soon as half the
            # casts have landed), then ref stages + casts + ref transposes.
            for h in range(2):
                bh = slice(h * (NB // 2), (h + 1) * (NB // 2))
                for ci in range(2):
                    nc.sync.dma_start_transpose(
                        out=pixT_bf[:, bh, slice(ci * P, (ci + 1) * P)],
                        in_=srcb_h[h][:, ci, :],
                    )
            for b in range(0, NB, 4):
                nc.vector.tensor_copy(
                    out=pixT_f8[:, b : b + 4, 0:C],
                    in_=pixT_bf[:, b : b + 4, 0:C],
                )
            for c in range(NS):
                st = early.tile([P, 2, SLICE], F32, name="rstage", tag="rst")
                (nc.sync if c % 2 == 0 else nc.scalar).dma_start(
                    out=st, in_=ref_r[:, :, ts(c, SLICE)]
                )
                ref_stages.append(st)
            for c in range(NS):
                nc.vector.tensor_copy(
                    out=refb_h[c // 4][:, :, ts(c % 4, SLICE)],
                    in_=ref_stages[c],
                )
            for h in range(2):
                bh = slice(h * (NB // 2), (h + 1) * (NB // 2))
                for ci in range(2):
                    nc.sync.dma_start_transpose(
                        out=pixT_bf[:, bh, slice(C + ci * P, C + (ci + 1) * P)],
                        in_=refb_h[h][:, ci, :],
                    )
            for b in range(0, NB, 4):
                nc.vector.tensor_copy(
                    out=pixT_f8[:, b : b + 4, C : 2 * C],
                    in_=pixT_bf[:, b : b + 4, C : 2 * C],
                )
            # blend precomputes: 1-m and m*ref
            nc.vector.tensor_scalar(
                out=omask_rep, in0=mask_rep, scalar1=-1.0, scalar2=1.0,
                op0=AluOpType.mult, op1=AluOpType.add,
            )
            for ci in range(2):
                for h in range(2):
                    jh = ts(h, HW // 2)
                    nc.vector.tensor_mul(
                        mref[:, ci, jh], mask_rep[:, jh], refb_h[h][:, ci, :]
                    )

        def f8slot(s, jb):
            return jb if jb < 4 * s else jb - 4

        with tc.tile_pool(name="fbuf", bufs=2) as fbuf, \
             tc.tile_pool(name="obuf", bufs=3) as obuf, \
             tc.tile_pool(name="zbuf", bufs=2) as zbuf:

            def emit_scores_group(s, g, f8, fbf):
                sl = ts(s, SLICE)
                pss = ps_sc.tile([P, 4, SLICE], F32, name="pss", tag="pss")
                for jp in range(2):
                    jb0, jb1 = 4 * g + 2 * jp, 4 * g + 2 * jp + 1
                    nc.tensor.matmul(
                        pss[:, 2 * jp, :], q2[0:CQ, ts(jb0, P)], q2[0:CQ, sl],
                        start=True, stop=True, tile_position=(0, 0),
                    )
                    nc.tensor.matmul(
                        pss[:, 2 * jp + 1, :], q2[CQ:P, ts(jb1, P)],
                        q2[CQ:P, sl],
                        start=True, stop=True, tile_position=(CQ, 0),
                    )
                if g == s:
                    nc.scalar.activation(
                        out=fbf, in_=pss, func=EXP, bias=exp_bias
                    )
                else:
                    fs = f8slot(s, 4 * g)
                    nc.scalar.activation(
                        out=f8[:, fs : fs + 4, :], in_=pss, func=EXP,
                        bias=exp_bias,
                    )

            def emit_z(s, f8, fbf):
                zps = ps_z.tile([2, SLICE], F32, name="psz", tag="psz")
                n_z = 0
                for g in range(NG):
                    if g == s:
                        for r in range(4):
                            nc.tensor.matmul(
                                zps, ones_bf, fbf[:, r, :],
                                start=(n_z == 0), stop=(n_z == 17),
                            )
                            n_z += 1
                    else:
                        fs = f8slot(s, 4 * g)
                        for r in range(2):
                            nc.tensor.matmul(
                                zps, ones8[:, :, 0:2],
                                f8[:, fs + 2 * r : fs + 2 * r + 2, :],
                                start=(n_z == 0), stop=(n_z == 17),
                                perf_mode=DR,
                            )
                            n_z += 1
                zinv_row = zbuf.tile([1, SLICE], F32, name="zr", tag="zr")
                invz_rep = zbuf.tile([P, SLICE], F32, name="zrep", tag="zrep")
                nc.vector.reciprocal_approx_fast(out=zinv_row, in_=zps[0:1, :])
                nc.gpsimd.dma_start(out=zrow[s], in_=zinv_row)
                nc.gpsimd.dma_start(
                    out=invz_rep, in_=zrow[s].partition_broadcast(P)
                )
                return invz_rep

            def emit_warm_chunk():
                wz = ps_z.tile([2, SLICE], F32, name="psz", tag="psz")
                for r in range(12):
                    nc.tensor.matmul(
                        wz, warm_sb[:, 0:2], warm_sb,
                        start=(r == 0), stop=(r == 11),
                    )

            def build_mm_list(s):
                """Ordered apply matmuls: one wave per output channel block
                (cb), each accumulating all 32 j-blocks into a 1-bank PSUM
                tile; start/stop flags per wave."""
                lst = []
                for cb in range(4):
                    blk = []
                    for g in range(NG):
                        if g == s:
                            for r in range(4):
                                blk.append([cb, g, r, False, 0, 0])
                        else:
                            for r in range(2):
                                blk.append([cb, g, r, True, 0, 0])
                    blk[0][4] = 1
                    blk[-1][5] = 1
                    lst.extend(blk)
                return lst

            def emit_apply_mm(ctx, e):
                cb, g, r, is_dr, st, sp = e
                s, f8, fbf, pso = ctx["s"], ctx["f8"], ctx["fbf"], ctx["pso"]
                cs = slice(cb * P, (cb + 1) * P)
                if is_dr:
                    fs = f8slot(s, 4 * g)
                    jb = 4 * g + 2 * r
                    nc.tensor.matmul(
                        pso, pixT_f8[:, jb : jb + 2, cs],
                        f8[:, fs + 2 * r : fs + 2 * r + 2, :],
                        start=bool(st), stop=bool(sp), perf_mode=DR,
                    )
                else:
                    jb = 4 * g + r
                    nc.tensor.matmul(
                        pso, pixT_bf[:, jb, cs], fbf[:, r, :],
                        start=bool(st), stop=bool(sp),
                    )

            def emit_finalize_wave(ctx, cb):
                s, invz_rep = ctx["s"], ctx["invz"]
                sl = ts(s, SLICE)
                outb = ctx["outb"]
                scr = obuf.tile([P, SLICE], F32, name="scr", tag="scr")
                nc.vector.tensor_copy(out=scr, in_=ctx["pso"])
                if cb < 2:
                    # src_att * invz -> out rows 256..512
                    nc.vector.tensor_mul(outb[:, 2 + cb, :], scr, invz_rep)
                else:
                    # flow = ref_att*invz*(1-m) + m*ref -> out rows 0..256
                    if cb == 2:
                        a_s = obuf.tile([P, SLICE], F32, name="a_s", tag="a_s")
                        nc.vector.tensor_mul(
                            a_s, omask_rep[:, sl], invz_rep
                        )
                        ctx["a_s"] = a_s
                    nc.vector.tensor_mul(
                        outb[:, cb - 2, :], scr, ctx["a_s"]
                    )
                    nc.vector.tensor_add(
                        outb[:, cb - 2, :], outb[:, cb - 2, :],
                        mref[:, cb - 2, sl],
                    )
                if cb == 3:
                    oq = [nc.sync, nc.gpsimd]
                    for k in range(4):
                        oq[(s + k) % 2].dma_start(
                            out=out_r[k, :, sl], in_=outb[:, k, :]
                        )

            prev = None
            for s in range(NS):
                f8 = fbuf.tile([P, NB - 4, SLICE], F8, name="f8", tag="f8")
                fbf = fbuf.tile([P, 4, SLICE], BF16, name="fbf", tag="fbf")
                ctx = {"s": s, "f8": f8, "fbf": fbf}
                # interleave this slice's scores/exp chain with the previous
                # slice's apply matmuls in chunks of 9 per score group, so
                # the PE always has work while the exp chain serializes on
                # its single PSUM group buffer.
                for g in range(NG):
                    emit_scores_group(s, g, f8, fbf)
                    if prev is not None:
                        if g == 0:
                            prev["outb"] = obuf.tile(
                                [P, 4, SLICE], F32, name="outb", tag="outb"
                            )
                        if g % 2 == 0:
                            prev["pso"] = ps_ap.tile(
                                [P, SLICE], F32, name="psa", tag="psa"
                            )
                        for e in prev["mm"][9 * g : 9 * g + 9]:
                            emit_apply_mm(prev, e)
                        if g % 2 == 1:
                            emit_finalize_wave(prev, g // 2)
                    elif g < 7:
                        emit_warm_chunk()
                ctx["invz"] = emit_z(s, f8, fbf)
                ctx["mm"] = build_mm_list(s)
                prev = ctx
            # drain: last slice's apply + finalize
            prev["outb"] = obuf.tile([P, 4, SLICE], F32, name="outb", tag="outb")
            for cb in range(4):
                prev["pso"] = ps_ap.tile([P, SLICE], F32, name="psa", tag="psa")
                for e in prev["mm"][18 * cb : 18 * cb + 18]:
                    emit_apply_mm(prev, e)
                emit_finalize_wave(prev, cb)
            # keep the HAM clock at 8/8 while the last finalize + output
            # DMAs drain (the blend + stores run ~2x slower at half clock)
            for k in range(4):
                emit_warm_chunk()


def build():
    nc = bacc.Bacc(
        "TRN2",
        target_bir_lowering=False,
        debug=False,
        enable_asserts=False,
        num_devices=NCORES,
    )
    src = nc.dram_tensor("src", (C, HW), F32, kind="ExternalInput")
    ref = nc.dram_tensor("ref", (C, HW), F32, kind="ExternalInput")
    mask = nc.dram_tensor("mask", (HW,), F32, kind="ExternalInput")
    wT = nc.dram_tensor("wT", (C, 2 * CQ), BF16, kind="ExternalInput")
    out = nc.dram_tensor("out", (2 * C, HW), F32, kind="ExternalOutput")
    with tile.TileContext(nc) as tc:
        _build_body(tc, src, ref, mask, wT, out)
    nc.compile()
    return nc


_CACHE = {}


def _get_nc():
    if "nc" not in _CACHE:
        _CACHE["nc"] = build()
    return _CACHE["nc"]


def _in_maps(src_mask, src_feature, ref_feature, conv_w):
    import ml_dtypes

    n_batch = src_feature.shape[0]
    wT1 = np.asarray(conv_w, dtype=np.float32).T.astype(ml_dtypes.bfloat16)
    # duplicated columns: the conv then writes q into BOTH partition halves
    # of q2 in one matmul (the scores pairs need q at partitions 0-63 and
    # 64-127 for tile_position packing)
    wT = np.ascontiguousarray(np.concatenate([wT1, wT1], axis=1))
    maps = []
    for n in range(n_batch):
        maps.append(
            {
                "src": np.ascontiguousarray(
                    np.asarray(src_feature[n], dtype=np.float32).reshape(C, HW)
                ),
                "ref": np.ascontiguousarray(
                    np.asarray(ref_feature[n], dtype=np.float32).reshape(C, HW)
                ),
                "mask": np.ascontiguousarray(
                    np.asarray(src_mask[n], dtype=np.float32).reshape(HW)
                ),
                "wT": wT,
            }
        )
    return maps


def _install_ntff_hook():
    """The agent image's antenv lacks axon_hooks; recreate it so
    run_bass_kernel_spmd(trace=True) can capture NTFF profiles."""
    import sys
    import types

    if "antenv.axon_hooks" in sys.modules:
        return
    import antenv
    from trn_agent_boot.trn_boot import _ntff_profile_via_ctypes

    hook = _ntff_profile_via_ctypes("/opt/axon/libaxon_pjrt.so")
    mod = types.ModuleType("antenv.axon_hooks")
    mod._hook = hook
    mod.set_axon_ntff_profile_hook = lambda h: setattr(mod, "_hook", h)
    mod.get_axon_ntff_profile_hook = lambda: mod._hook
    sys.modules["antenv.axon_hooks"] = mod
    antenv.axon_hooks = mod


def run(src_mask, src_feature, ref_feature, conv_w, trace=False):
    """Run on 8 NeuronCores. Returns (output [N,2C,H,W], BassKernelResults)."""
    n_batch, c, h, w = src_feature.shape
    if trace:
        _install_ntff_hook()
    nc = _get_nc()
    maps = _in_maps(src_mask, src_feature, ref_feature, conv_w)
    res = bass_utils.run_bass_kernel_spmd(
        nc, maps, core_ids=list(range(NCORES)), trace=trace
    )
    out = np.stack([r["out"] for r in res.results], axis=0)
    return out.reshape(n_batch, 2 * c, h, w).astype(np.float32), res


def kernel(src_mask, src_feature, ref_feature, conv_w):
    out, _ = run(src_mask, src_feature, ref_feature, conv_w)
    return out


# revision 27
# speedup vs baseline: 1.0237x; 1.0237x over previous
"""Trainium2 Bass kernel for ExampleGuidedAttention (N=8, C=256, H=W=64).

Data-parallel over batch N across 8 NeuronCores; each core computes one
batch element's full guided attention.

Algorithm notes (per core):
  q = conv_w @ src_pix                      [64, 4096]   (PE, bf16)
  S^T[j,i] = sum_o q[o,j] q[o,i]            (PE, bf16; S symmetric; two
             j-blocks packed in the 128x128 array via tile_position)
  F[j,i] = exp(S^T[j,i] - 64 + 13*ln2)      (ACT; global shift keeps the
             fp32 exp in range; the 2^13 factor cancels against 1/Z)
  Per column-slice s (512 pixels) the F tiles span ALL j, so
  Z[i] = sum_j F[j,i] is computed per-slice with ones-vector matmuls on
  the PE, and each slice normalizes + blends + stores immediately --
  no full-image unnormalized buffer and no end-of-kernel fixup tail.

  Off-diagonal j-block tiles (28 of 32 per slice) are stored in fp8-e4m3
  and applied with DoubleRow matmuls (2 j-blocks per pass, 2x PE rate).
  The 4 diagonal-crossing tiles stay bf16 so the dominant near-diagonal
  attention terms keep full precision; all tiles share the 2^13 scale so
  they accumulate consistently in PSUM and the scale cancels in 1/Z.

  out = [ (1-m)*ref_att*invZ + m*ref ; src_att*invZ ]

The issue order software-pipelines slice s's scores/exp chain (ACT
bound, single PSUM group buffer) against slice s-1's apply matmuls so
the PE never stalls on the exp chain.
"""

import math

import numpy as np

import concourse.bass as bass
import concourse.mybir as mybir
import concourse.tile as tile
from concourse import bacc, bass_utils
from concourse.bass import ts
from concourse.alu_op_type import AluOpType

P = 128
C = 256          # feature channels
CQ = 64          # query channels
HW = 4096        # pixels per image
NB = HW // P     # 32 pixel blocks (contraction chunks)
SLICE = 512
NS = HW // SLICE  # 8 output column slices
NG = NB // 4      # 8 score groups of 4 j-blocks per slice
NCORES = 8
K_SCALE = 13.0    # F scaled by 2^13: off-diag fp8 overflow headroom to S=70

F32 = mybir.dt.float32
BF16 = mybir.dt.bfloat16
F8 = mybir.dt.float8e4
EXP = mybir.ActivationFunctionType.Exp
DR = mybir.MatmulPerfMode.DoubleRow


def _build_body(tc, src, ref, mask, wT, out):
    nc = tc.nc
    src_r = src.ap().rearrange("(ci p) j -> p ci j", p=P)   # [128, 2, 4096]
    ref_r = ref.ap().rearrange("(ci p) j -> p ci j", p=P)
    wT_r = wT.ap().rearrange("(ci p) o -> p ci o", p=P)     # [128, 2, 128]
    out_r = out.ap().rearrange("(cb p) j -> cb p j", p=P)   # [4, 128, 4096]

    with (
        tc.tile_pool(name="persist", bufs=1) as persist,
        tc.tile_pool(name="ps_sc", bufs=1, space="PSUM") as ps_sc,
        tc.tile_pool(name="ps_ap", bufs=2, space="PSUM") as ps_ap,
        tc.tile_pool(name="ps_z", bufs=1, space="PSUM") as ps_z,
        tc.tile_pool(name="dram", bufs=1, space="DRAM") as dram,
    ):
        refb_h = [
            persist.tile([P, 2, HW // 2], BF16, name=f"refb{h}")
            for h in range(2)
        ]
        q2 = persist.tile([P, HW], BF16)
        pixT_bf = persist.tile([P, NB, 2 * C], BF16)   # [src 256 | ref 256]
        pixT_f8 = persist.tile([P, NB, 2 * C], F8)
        wT_sb = persist.tile([P, 2, 2 * CQ], BF16)
        mask_rep = persist.tile([P, HW], BF16)
        omask_rep = persist.tile([P, HW], BF16)        # 1 - mask
        mref = persist.tile([P, 2, HW], BF16)          # mask * ref
        exp_bias = persist.tile([P, 1], F32)
        ones8 = persist.tile([P, 2, 16], F8)  # 16B k-pair stride for dual-fp8 ldweights
        ones_bf = persist.tile([P, 2], BF16)
        warm_sb = persist.tile([P, SLICE], BF16)
        zrow = dram.tile([NS, SLICE], F32)
        nc.vector.memset(exp_bias, -64.0 + K_SCALE * math.log(2.0))
        nc.vector.memset(ones8, 1.0)
        nc.vector.memset(ones_bf, 1.0)
        nc.vector.memset(warm_sb, 0.0)

        nc.sync.dma_start(out=wT_sb, in_=wT_r)
        for s in range(NS):
            nc.gpsimd.dma_start(
                out=mask_rep[:, ts(s, SLICE)],
                in_=mask.ap()[ts(s, SLICE)].partition_broadcast(P),
            )

        with tc.tile_pool(name="early", bufs=6) as early, \
             tc.tile_pool(name="early1", bufs=1) as early1:
            # PE warmup: back-to-back matmuls on zeroed data keep the HAM
            # clock gate at 8/8 (2.4 GHz) while input DMAs stream in.
            warm_ps = ps_sc.tile([P, 4, SLICE], F32, name="pss", tag="pss")
            for r in range(20):
                nc.tensor.matmul(
                    warm_ps[:, r % 4, :], warm_sb[:, 0:P], warm_sb,
                    start=True, stop=True,
                )
            # warm the ACT exp table so slice 0 doesn't pay the table load
            nc.scalar.activation(
                out=warm_sb[:, 0:1], in_=warm_sb[:, 0:1], func=EXP,
                bias=exp_bias,
            )

            srcb_h = [
                early1.tile([P, 2, HW // 2], BF16, name=f"srcb{h}")
                for h in range(2)
            ]
            # fp32 inputs via fast hardware DMA.  src casts on DVE (conv ->
            # q gates everything); ref casts on ACT so the DVE can proceed
            # to q2 / fp8 work without head-of-line blocking.
            # src in 8 0.5MB chunks: each dma_start lands on its own
            # hardware queue (~85GB/s each).  ref is issued on sync AFTER
            # the src transposes so src gets the full HBM read bandwidth
            # first (conv -> q -> scores gates everything; ref is needed
            # ~30us later).
            src_stages, ref_stages = [], []
            SC = SLICE // 2
            for c in range(16):
                st = early.tile([P, 2, SC], F32, name="stage", tag="st")
                (nc.sync if c % 2 == 0 else nc.scalar).dma_start(
                    out=st, in_=src_r[:, :, ts(c, SC)]
                )
                src_stages.append(st)
            for c in range(16):
                nc.vector.tensor_copy(
                    out=srcb_h[c // 8][:, :, ts(c % 8, SC)],
                    in_=src_stages[c],
                )
            # 1x1 conv: q = wT.T @ src_pix; q into both partition halves.
            # 3 warm matmuls between slices keep the PE (and HAM clock)
            # busy while the next slice's DMA+cast lands.
            for s in range(NS):
                sl = ts(s, SLICE)
                psq = ps_z.tile([P, SLICE], F32, name="psz", tag="psz")
                for ci in range(2):
                    nc.tensor.matmul(
                        psq,
                        wT_sb[:, ci, :],
                        srcb_h[s // 4][:, ci, ts(s % 4, SLICE)],
                        start=(ci == 0),
                        stop=(ci == 1),
                    )
                for r in range(3):
                    nc.tensor.matmul(
                        warm_ps[:, r, :], warm_sb[:, 0:P], warm_sb,
                        start=True, stop=True,
                    )
                nc.vector.tensor_copy(out=q2[:, sl], in_=psq)
            # XBAR transposes: pixT[p, b, c] = pix[c, b*128+p]; src halves
            # first (the j-half split lets each start as soon as half the
            # casts have landed), then ref stages + casts + ref transposes.
            for h in range(2):
                bh = slice(h * (NB // 2), (h + 1) * (NB // 2))
                for ci in range(2):
                    nc.sync.dma_start_transpose(
                        out=pixT_bf[:, bh, slice(ci * P, (ci + 1) * P)],
                        in_=srcb_h[h][:, ci, :],
                    )
            for b in range(0, NB, 4):
                nc.vector.tensor_copy(
                    out=pixT_f8[:, b : b + 4, 0:C],
                    in_=pixT_bf[:, b : b + 4, 0:C],
                )
            for c in range(NS):
                st = early.tile([P, 2, SLICE], F32, name="rstage", tag="rst")
                nc.sync.dma_start(out=st, in_=ref_r[:, :, ts(c, SLICE)])
                ref_stages.append(st)
            for c in range(NS):
                nc.vector.tensor_copy(
                    out=refb_h[c // 4][:, :, ts(c % 4, SLICE)],
                    in_=ref_stages[c],
                )
            for h in range(2):
                bh = slice(h * (NB // 2), (h + 1) * (NB // 2))
                for ci in range(2):
                    nc.sync.dma_start_transpose(
                        out=pixT_bf[:, bh, slice(C + ci * P, C + (ci + 1) * P)],
                        in_=refb_h[h][:, ci, :],
                    )
            for b in range(0, NB, 4):
                nc.vector.tensor_copy(
                    out=pixT_f8[:, b : b + 4, C : 2 * C],
                    in_=pixT_bf[:, b : b + 4, C : 2 * C],
                )
            # blend precomputes: 1-m and m*ref
            nc.vector.tensor_scalar(
                out=omask_rep, in0=mask_rep, scalar1=-1.0, scalar2=1.0,
                op0=AluOpType.mult, op1=AluOpType.add,
            )
            for ci in range(2):
                for h in range(2):
                    jh = ts(h, HW // 2)
                    nc.vector.tensor_mul(
                        mref[:, ci, jh], mask_rep[:, jh], refb_h[h][:, ci, :]
                    )

        def f8slot(s, jb):
            return jb if jb < 4 * s else jb - 4

        with tc.tile_pool(name="fbuf", bufs=2) as fbuf, \
             tc.tile_pool(name="obuf", bufs=3) as obuf, \
             tc.tile_pool(name="zbuf", bufs=2) as zbuf:

            def emit_scores_group(s, g, f8, fbf):
                sl = ts(s, SLICE)
                pss = ps_sc.tile([P, 4, SLICE], F32, name="pss", tag="pss")
                for jp in range(2):
                    jb0, jb1 = 4 * g + 2 * jp, 4 * g + 2 * jp + 1
                    nc.tensor.matmul(
                        pss[:, 2 * jp, :], q2[0:CQ, ts(jb0, P)], q2[0:CQ, sl],
                        start=True, stop=True, tile_position=(0, 0),
                    )
                    nc.tensor.matmul(
                        pss[:, 2 * jp + 1, :], q2[CQ:P, ts(jb1, P)],
                        q2[CQ:P, sl],
                        start=True, stop=True, tile_position=(CQ, 0),
                    )
                if g == s:
                    nc.scalar.activation(
                        out=fbf, in_=pss, func=EXP, bias=exp_bias
                    )
                else:
                    fs = f8slot(s, 4 * g)
                    nc.scalar.activation(
                        out=f8[:, fs : fs + 4, :], in_=pss, func=EXP,
                        bias=exp_bias,
                    )

            def emit_z(s, f8, fbf):
                zps = ps_z.tile([2, SLICE], F32, name="psz", tag="psz")
                n_z = 0
                for g in range(NG):
                    if g == s:
                        for r in range(4):
                            nc.tensor.matmul(
                                zps, ones_bf, fbf[:, r, :],
                                start=(n_z == 0), stop=(n_z == 17),
                            )
                            n_z += 1
                    else:
                        fs = f8slot(s, 4 * g)
                        for r in range(2):
                            nc.tensor.matmul(
                                zps, ones8[:, :, 0:2],
                                f8[:, fs + 2 * r : fs + 2 * r + 2, :],
                                start=(n_z == 0), stop=(n_z == 17),
                                perf_mode=DR,
                            )
                            n_z += 1
                zinv_row = zbuf.tile([1, SLICE], F32, name="zr", tag="zr")
                invz_rep = zbuf.tile([P, SLICE], F32, name="zrep", tag="zrep")
                nc.vector.reciprocal_approx_fast(out=zinv_row, in_=zps[0:1, :])
                nc.gpsimd.dma_start(out=zrow[s], in_=zinv_row)
                nc.gpsimd.dma_start(
                    out=invz_rep, in_=zrow[s].partition_broadcast(P)
                )
                return invz_rep

            def emit_warm_chunk():
                wz = ps_z.tile([2, SLICE], F32, name="psz", tag="psz")
                for r in range(12):
                    nc.tensor.matmul(
                        wz, warm_sb[:, 0:2], warm_sb,
                        start=(r == 0), stop=(r == 11),
                    )

            def build_mm_list(s):
                """Ordered apply matmuls: one wave per output channel block
                (cb), each accumulating all 32 j-blocks into a 1-bank PSUM
                tile; start/stop flags per wave."""
                lst = []
                for cb in range(4):
                    blk = []
                    for g in range(NG):
                        if g == s:
                            for r in range(4):
                                blk.append([cb, g, r, False, 0, 0])
                        else:
                            for r in range(2):
                                blk.append([cb, g, r, True, 0, 0])
                    blk[0][4] = 1
                    blk[-1][5] = 1
                    lst.extend(blk)
                return lst

            def emit_apply_mm(ctx, e):
                cb, g, r, is_dr, st, sp = e
                s, f8, fbf, pso = ctx["s"], ctx["f8"], ctx["fbf"], ctx["pso"]
                cs = slice(cb * P, (cb + 1) * P)
                if is_dr:
                    fs = f8slot(s, 4 * g)
                    jb = 4 * g + 2 * r
                    nc.tensor.matmul(
                        pso, pixT_f8[:, jb : jb + 2, cs],
                        f8[:, fs + 2 * r : fs + 2 * r + 2, :],
                        start=bool(st), stop=bool(sp), perf_mode=DR,
                    )
                else:
                    jb = 4 * g + r
                    nc.tensor.matmul(
                        pso, pixT_bf[:, jb, cs], fbf[:, r, :],
                        start=bool(st), stop=bool(sp),
                    )

            def emit_finalize_wave(ctx, cb):
                s, invz_rep = ctx["s"], ctx["invz"]
                sl = ts(s, SLICE)
                outb = ctx["outb"]
                scr = obuf.tile([P, SLICE], F32, name="scr", tag="scr")
                nc.vector.tensor_copy(out=scr, in_=ctx["pso"])
                if cb < 2:
                    # src_att * invz -> out rows 256..512
                    nc.vector.tensor_mul(outb[:, 2 + cb, :], scr, invz_rep)
                else:
                    # flow = ref_att*invz*(1-m) + m*ref -> out rows 0..256
                    if cb == 2:
                        a_s = obuf.tile([P, SLICE], F32, name="a_s", tag="a_s")
                        nc.vector.tensor_mul(
                            a_s, omask_rep[:, sl], invz_rep
                        )
                        ctx["a_s"] = a_s
                    nc.vector.tensor_mul(
                        outb[:, cb - 2, :], scr, ctx["a_s"]
                    )
                    nc.vector.tensor_add(
                        outb[:, cb - 2, :], outb[:, cb - 2, :],
                        mref[:, cb - 2, sl],
                    )
                if cb == 3:
                    oq = [nc.sync, nc.gpsimd]
                    for k in range(4):
                        oq[(s + k) % 2].dma_start(
                            out=out_r[k, :, sl], in_=outb[:, k, :]
                        )

            prev = None
            for s in range(NS):
                f8 = fbuf.tile([P, NB - 4, SLICE], F8, name="f8", tag="f8")
                fbf = fbuf.tile([P, 4, SLICE], BF16, name="fbf", tag="fbf")
                ctx = {"s": s, "f8": f8, "fbf": fbf}
                # interleave this slice's scores/exp chain with the previous
                # slice's apply matmuls in chunks of 9 per score group, so
                # the PE always has work while the exp chain serializes on
                # its single PSUM group buffer.
                for g in range(NG):
                    emit_scores_group(s, g, f8, fbf)
                    if prev is not None:
                        if g == 0:
                            prev["outb"] = obuf.tile(
                                [P, 4, SLICE], F32, name="outb", tag="outb"
                            )
                        if g % 2 == 0:
                            prev["pso"] = ps_ap.tile(
                                [P, SLICE], F32, name="psa", tag="psa"
                            )
                        for e in prev["mm"][9 * g : 9 * g + 9]:
                            emit_apply_mm(prev, e)
                        if g % 2 == 1:
                            emit_finalize_wave(prev, g // 2)
                    elif g < 7:
                        emit_warm_chunk()
                ctx["invz"] = emit_z(s, f8, fbf)
                ctx["mm"] = build_mm_list(s)
                prev = ctx
            # drain: last slice's apply + finalize
            prev["outb"] = obuf.tile([P, 4, SLICE], F32, name="outb", tag="outb")
            for cb in range(4):
                prev["pso"] = ps_ap.tile([P, SLICE], F32, name="psa", tag="psa")
                for e in prev["mm"][18 * cb : 18 * cb + 18]:
                    emit_apply_mm(prev, e)
                emit_finalize_wave(prev, cb)
            # keep the HAM clock at 8/8 while the last finalize + output
            # DMAs drain (the blend + stores run ~2x slower at half clock)
            for k in range(4):
                emit_warm_chunk()


def build():
    nc = bacc.Bacc(
        "TRN2",
        target_bir_lowering=False,
        debug=False,
        enable_asserts=False,
        num_devices=NCORES,
    )
    src = nc.dram_tensor("src", (C, HW), F32, kind="ExternalInput")
    ref = nc.dram_tensor("ref", (C, HW), F32, kind="ExternalInput")
    mask = nc.dram_tensor("mask", (HW,), F32, kind="ExternalInput")
    wT = nc.dram_tensor("wT", (C, 2 * CQ), BF16, kind="ExternalInput")
    out = nc.dram_tensor("out", (2 * C, HW), F32, kind="ExternalOutput")
    with tile.TileContext(nc) as tc:
        _build_body(tc, src, ref, mask, wT, out)
    nc.compile()
    return nc


_CACHE = {}


def _get_nc():
    if "nc" not in _CACHE:
        _CACHE["nc"] = build()
    return _CACHE["nc"]


def _in_maps(src_mask, src_feature, ref_feature, conv_w):
    import ml_dtypes

    n_batch = src_feature.shape[0]
    wT1 = np.asarray(conv_w, dtype=np.float32).T.astype(ml_dtypes.bfloat16)
    # duplicated columns: the conv then writes q into BOTH partition halves
    # of q2 in one matmul (the scores pairs need q at partitions 0-63 and
    # 64-127 for tile_position packing)
    wT = np.ascontiguousarray(np.concatenate([wT1, wT1], axis=1))
    maps = []
    for n in range(n_batch):
        maps.append(
            {
                "src": np.ascontiguousarray(
                    np.asarray(src_feature[n], dtype=np.float32).reshape(C, HW)
                ),
                "ref": np.ascontiguousarray(
                    np.asarray(ref_feature[n], dtype=np.float32).reshape(C, HW)
                ),
                "mask": np.ascontiguousarray(
                    np.asarray(src_mask[n], dtype=np.float32).reshape(HW)
                ),
                "wT": wT,
            }
        )
    return maps


def _install_ntff_hook():
    """The agent image's antenv lacks axon_hooks; recreate it so
    run_bass_kernel_spmd(trace=True) can capture NTFF profiles."""
    import sys
    import types

    if "antenv.axon_hooks" in sys.modules:
        return
    import antenv
    from trn_agent_boot.trn_boot import _ntff_profile_via_ctypes

    hook = _ntff_profile_via_ctypes("/opt/axon/libaxon_pjrt.so")
    mod = types.ModuleType("antenv.axon_hooks")
    mod._hook = hook
    mod.set_axon_ntff_profile_hook = lambda h: setattr(mod, "_hook", h)
    mod.get_axon_ntff_profile_hook = lambda: mod._hook
    sys.modules["antenv.axon_hooks"] = mod
    antenv.axon_hooks = mod


def run(src_mask, src_feature, ref_feature, conv_w, trace=False):
    """Run on 8 NeuronCores. Returns (output [N,2C,H,W], BassKernelResults)."""
    n_batch, c, h, w = src_feature.shape
    if trace:
        _install_ntff_hook()
    nc = _get_nc()
    maps = _in_maps(src_mask, src_feature, ref_feature, conv_w)
    res = bass_utils.run_bass_kernel_spmd(
        nc, maps, core_ids=list(range(NCORES)), trace=trace
    )
    out = np.stack([r["out"] for r in res.results], axis=0)
    return out.reshape(n_batch, 2 * c, h, w).astype(np.float32), res


def kernel(src_mask, src_feature, ref_feature, conv_w):
    out, _ = run(src_mask, src_feature, ref_feature, conv_w)
    return out


# revision 28
# speedup vs baseline: 1.0538x; 1.0294x over previous
"""Trainium2 Bass kernel for ExampleGuidedAttention (N=8, C=256, H=W=64).

Data-parallel over batch N across 8 NeuronCores; each core computes one
batch element's full guided attention.

Algorithm notes (per core):
  q = conv_w @ src_pix                      [64, 4096]   (PE, bf16)
  S^T[j,i] = sum_o q[o,j] q[o,i]            (PE, bf16; S symmetric; two
             j-blocks packed in the 128x128 array via tile_position)
  F[j,i] = exp(S^T[j,i] - 64 + 13*ln2)      (ACT; global shift keeps the
             fp32 exp in range; the 2^13 factor cancels against 1/Z)
  Per column-slice s (512 pixels) the F tiles span ALL j, so
  Z[i] = sum_j F[j,i] is computed per-slice with ones-vector matmuls on
  the PE, and each slice normalizes + blends + stores immediately --
  no full-image unnormalized buffer and no end-of-kernel fixup tail.

  Off-diagonal j-block tiles (28 of 32 per slice) are stored in fp8-e4m3
  and applied with DoubleRow matmuls (2 j-blocks per pass, 2x PE rate).
  The 4 diagonal-crossing tiles stay bf16 so the dominant near-diagonal
  attention terms keep full precision; all tiles share the 2^13 scale so
  they accumulate consistently in PSUM and the scale cancels in 1/Z.

  out = [ (1-m)*ref_att*invZ + m*ref ; src_att*invZ ]

The issue order software-pipelines slice s's scores/exp chain (ACT
bound, single PSUM group buffer) against slice s-1's apply matmuls so
the PE never stalls on the exp chain.
"""

import math

import numpy as np

import concourse.bass as bass
import concourse.mybir as mybir
import concourse.tile as tile
from concourse import bacc, bass_utils
from concourse.bass import ts
from concourse.alu_op_type import AluOpType

P = 128
C = 256          # feature channels
CQ = 64          # query channels
HW = 4096        # pixels per image
NB = HW // P     # 32 pixel blocks (contraction chunks)
SLICE = 512
NS = HW // SLICE  # 8 output column slices
NG = NB // 4      # 8 score groups of 4 j-blocks per slice
NCORES = 8
K_SCALE = 13.0    # F scaled by 2^13: off-diag fp8 overflow headroom to S=70

F32 = mybir.dt.float32
BF16 = mybir.dt.bfloat16
F8 = mybir.dt.float8e4
EXP = mybir.ActivationFunctionType.Exp
DR = mybir.MatmulPerfMode.DoubleRow


def _build_body(tc, src, ref, mask, wT, out):
    nc = tc.nc
    src_r = src.ap().rearrange("(ci p) j -> p ci j", p=P)   # [128, 2, 4096]
    ref_r = ref.ap().rearrange("(ci p) j -> p ci j", p=P)
    wT_r = wT.ap().rearrange("(ci p) o -> p ci o", p=P)     # [128, 2, 128]
    out_r = out.ap().rearrange("(cb p) j -> cb p j", p=P)   # [4, 128, 4096]

    with (
        tc.tile_pool(name="persist", bufs=1) as persist,
        tc.tile_pool(name="ps_sc", bufs=1, space="PSUM") as ps_sc,
        tc.tile_pool(name="ps_ap", bufs=3, space="PSUM") as ps_ap,
        tc.tile_pool(name="ps_z", bufs=1, space="PSUM") as ps_z,
        tc.tile_pool(name="dram", bufs=1, space="DRAM") as dram,
    ):
        refb_h = [
            persist.tile([P, 2, HW // 2], BF16, name=f"refb{h}")
            for h in range(2)
        ]
        q2 = persist.tile([P, HW], BF16)
        pixT_bf = persist.tile([P, NB, 2 * C], BF16)   # [src 256 | ref 256]
        pixT_f8 = persist.tile([P, NB, 2 * C], F8)
        wT_sb = persist.tile([P, 2, 2 * CQ], BF16)
        mask_rep = persist.tile([P, HW], BF16)
        omask_rep = persist.tile([P, HW], BF16)        # 1 - mask
        mref = persist.tile([P, 2, HW], BF16)          # mask * ref
        exp_bias = persist.tile([P, 1], F32)
        ones8 = persist.tile([P, 2, 16], F8)  # 16B k-pair stride for dual-fp8 ldweights
        ones_bf = persist.tile([P, 2], BF16)
        warm_sb = persist.tile([P, SLICE], BF16)
        zrow = dram.tile([NS, SLICE], F32)
        nc.vector.memset(exp_bias, -64.0 + K_SCALE * math.log(2.0))
        nc.vector.memset(ones8, 1.0)
        nc.vector.memset(ones_bf, 1.0)
        nc.vector.memset(warm_sb, 0.0)

        nc.sync.dma_start(out=wT_sb, in_=wT_r)
        for s in range(NS):
            nc.gpsimd.dma_start(
                out=mask_rep[:, ts(s, SLICE)],
                in_=mask.ap()[ts(s, SLICE)].partition_broadcast(P),
            )

        with tc.tile_pool(name="early", bufs=6) as early, \
             tc.tile_pool(name="early1", bufs=1) as early1:
            # PE warmup: back-to-back matmuls on zeroed data keep the HAM
            # clock gate at 8/8 (2.4 GHz) while input DMAs stream in.
            warm_ps = ps_sc.tile([P, 4, SLICE], F32, name="pss", tag="pss")
            for r in range(20):
                nc.tensor.matmul(
                    warm_ps[:, r % 4, :], warm_sb[:, 0:P], warm_sb,
                    start=True, stop=True,
                )
            # warm the ACT exp table so slice 0 doesn't pay the table load
            nc.scalar.activation(
                out=warm_sb[:, 0:1], in_=warm_sb[:, 0:1], func=EXP,
                bias=exp_bias,
            )

            srcb_h = [
                early1.tile([P, 2, HW // 2], BF16, name=f"srcb{h}")
                for h in range(2)
            ]
            # fp32 inputs via fast hardware DMA.  src casts on DVE (conv ->
            # q gates everything); ref casts on ACT so the DVE can proceed
            # to q2 / fp8 work without head-of-line blocking.
            # src in 8 0.5MB chunks: each dma_start lands on its own
            # hardware queue (~85GB/s each).  ref is issued on sync AFTER
            # the src transposes so src gets the full HBM read bandwidth
            # first (conv -> q -> scores gates everything; ref is needed
            # ~30us later).
            src_stages, ref_stages = [], []
            SC = SLICE // 2
            for c in range(16):
                st = early.tile([P, 2, SC], F32, name="stage", tag="st")
                (nc.sync if c % 2 == 0 else nc.scalar).dma_start(
                    out=st, in_=src_r[:, :, ts(c, SC)]
                )
                src_stages.append(st)
            for c in range(16):
                nc.vector.tensor_copy(
                    out=srcb_h[c // 8][:, :, ts(c % 8, SC)],
                    in_=src_stages[c],
                )
            # 1x1 conv: q = wT.T @ src_pix; q into both partition halves.
            # 3 warm matmuls between slices keep the PE (and HAM clock)
            # busy while the next slice's DMA+cast lands.
            for s in range(NS):
                sl = ts(s, SLICE)
                psq = ps_z.tile([P, SLICE], F32, name="psz", tag="psz")
                for ci in range(2):
                    nc.tensor.matmul(
                        psq,
                        wT_sb[:, ci, :],
                        srcb_h[s // 4][:, ci, ts(s % 4, SLICE)],
                        start=(ci == 0),
                        stop=(ci == 1),
                    )
                for r in range(3):
                    nc.tensor.matmul(
                        warm_ps[:, r, :], warm_sb[:, 0:P], warm_sb,
                        start=True, stop=True,
                    )
                nc.vector.tensor_copy(out=q2[:, sl], in_=psq)
            # XBAR transposes: pixT[p, b, c] = pix[c, b*128+p]; src halves
            # first (the j-half split lets each start as soon as half the
            # casts have landed), then ref stages + casts + ref transposes.
            for h in range(2):
                bh = slice(h * (NB // 2), (h + 1) * (NB // 2))
                for ci in range(2):
                    nc.sync.dma_start_transpose(
                        out=pixT_bf[:, bh, slice(ci * P, (ci + 1) * P)],
                        in_=srcb_h[h][:, ci, :],
                    )
            for b in range(0, NB, 4):
                nc.vector.tensor_copy(
                    out=pixT_f8[:, b : b + 4, 0:C],
                    in_=pixT_bf[:, b : b + 4, 0:C],
                )
            for c in range(NS):
                st = early.tile([P, 2, SLICE], F32, name="rstage", tag="rst")
                nc.sync.dma_start(out=st, in_=ref_r[:, :, ts(c, SLICE)])
                ref_stages.append(st)
            for c in range(NS):
                nc.vector.tensor_copy(
                    out=refb_h[c // 4][:, :, ts(c % 4, SLICE)],
                    in_=ref_stages[c],
                )
            for h in range(2):
                bh = slice(h * (NB // 2), (h + 1) * (NB // 2))
                for ci in range(2):
                    nc.sync.dma_start_transpose(
                        out=pixT_bf[:, bh, slice(C + ci * P, C + (ci + 1) * P)],
                        in_=refb_h[h][:, ci, :],
                    )
            for b in range(0, NB, 4):
                nc.vector.tensor_copy(
                    out=pixT_f8[:, b : b + 4, C : 2 * C],
                    in_=pixT_bf[:, b : b + 4, C : 2 * C],
                )
            # blend precomputes: 1-m and m*ref
            nc.vector.tensor_scalar(
                out=omask_rep, in0=mask_rep, scalar1=-1.0, scalar2=1.0,
                op0=AluOpType.mult, op1=AluOpType.add,
            )
            for ci in range(2):
                for h in range(2):
                    jh = ts(h, HW // 2)
                    nc.vector.tensor_mul(
                        mref[:, ci, jh], mask_rep[:, jh], refb_h[h][:, ci, :]
                    )

        def f8slot(s, jb):
            return jb if jb < 4 * s else jb - 4

        with tc.tile_pool(name="fbuf", bufs=2) as fbuf, \
             tc.tile_pool(name="obuf", bufs=3) as obuf, \
             tc.tile_pool(name="zbuf", bufs=2) as zbuf:

            def emit_scores_group(s, g, f8, fbf):
                sl = ts(s, SLICE)
                pss = ps_sc.tile([P, 4, SLICE], F32, name="pss", tag="pss")
                for jp in range(2):
                    jb0, jb1 = 4 * g + 2 * jp, 4 * g + 2 * jp + 1
                    nc.tensor.matmul(
                        pss[:, 2 * jp, :], q2[0:CQ, ts(jb0, P)], q2[0:CQ, sl],
                        start=True, stop=True, tile_position=(0, 0),
                    )
                    nc.tensor.matmul(
                        pss[:, 2 * jp + 1, :], q2[CQ:P, ts(jb1, P)],
                        q2[CQ:P, sl],
                        start=True, stop=True, tile_position=(CQ, 0),
                    )
                if g == s:
                    nc.scalar.activation(
                        out=fbf, in_=pss, func=EXP, bias=exp_bias
                    )
                else:
                    fs = f8slot(s, 4 * g)
                    nc.scalar.activation(
                        out=f8[:, fs : fs + 4, :], in_=pss, func=EXP,
                        bias=exp_bias,
                    )

            def emit_z(s, f8, fbf):
                zps = ps_z.tile([2, SLICE], F32, name="psz", tag="psz")
                n_z = 0
                for g in range(NG):
                    if g == s:
                        for r in range(4):
                            nc.tensor.matmul(
                                zps, ones_bf, fbf[:, r, :],
                                start=(n_z == 0), stop=(n_z == 17),
                            )
                            n_z += 1
                    else:
                        fs = f8slot(s, 4 * g)
                        for r in range(2):
                            nc.tensor.matmul(
                                zps, ones8[:, :, 0:2],
                                f8[:, fs + 2 * r : fs + 2 * r + 2, :],
                                start=(n_z == 0), stop=(n_z == 17),
                                perf_mode=DR,
                            )
                            n_z += 1
                zinv_row = zbuf.tile([1, SLICE], F32, name="zr", tag="zr")
                invz_rep = zbuf.tile([P, SLICE], F32, name="zrep", tag="zrep")
                nc.vector.reciprocal_approx_fast(out=zinv_row, in_=zps[0:1, :])
                nc.gpsimd.dma_start(out=zrow[s], in_=zinv_row)
                nc.gpsimd.dma_start(
                    out=invz_rep, in_=zrow[s].partition_broadcast(P)
                )
                return invz_rep

            def emit_warm_chunk():
                wz = ps_z.tile([2, SLICE], F32, name="psz", tag="psz")
                for r in range(12):
                    nc.tensor.matmul(
                        wz, warm_sb[:, 0:2], warm_sb,
                        start=(r == 0), stop=(r == 11),
                    )

            def build_mm_list(s):
                """Ordered apply matmuls: one wave per output channel block
                (cb), each accumulating all 32 j-blocks into a 1-bank PSUM
                tile; start/stop flags per wave."""
                lst = []
                for cb in range(4):
                    blk = []
                    for g in range(NG):
                        if g == s:
                            for r in range(4):
                                blk.append([cb, g, r, False, 0, 0])
                        else:
                            for r in range(2):
                                blk.append([cb, g, r, True, 0, 0])
                    blk[0][4] = 1
                    blk[-1][5] = 1
                    lst.extend(blk)
                return lst

            def emit_apply_mm(ctx, e):
                cb, g, r, is_dr, st, sp = e
                s, f8, fbf, pso = ctx["s"], ctx["f8"], ctx["fbf"], ctx["pso"]
                cs = slice(cb * P, (cb + 1) * P)
                if is_dr:
                    fs = f8slot(s, 4 * g)
                    jb = 4 * g + 2 * r
                    nc.tensor.matmul(
                        pso, pixT_f8[:, jb : jb + 2, cs],
                        f8[:, fs + 2 * r : fs + 2 * r + 2, :],
                        start=bool(st), stop=bool(sp), perf_mode=DR,
                    )
                else:
                    jb = 4 * g + r
                    nc.tensor.matmul(
                        pso, pixT_bf[:, jb, cs], fbf[:, r, :],
                        start=bool(st), stop=bool(sp),
                    )

            def emit_finalize_wave(ctx, cb):
                s, invz_rep = ctx["s"], ctx["invz"]
                sl = ts(s, SLICE)
                outb = ctx["outb"]
                scr = obuf.tile([P, SLICE], F32, name="scr", tag="scr")
                nc.vector.tensor_copy(out=scr, in_=ctx["pso"])
                if cb < 2:
                    # src_att * invz -> out rows 256..512
                    nc.vector.tensor_mul(outb[:, 2 + cb, :], scr, invz_rep)
                else:
                    # flow = ref_att*invz*(1-m) + m*ref -> out rows 0..256
                    if cb == 2:
                        a_s = obuf.tile([P, SLICE], F32, name="a_s", tag="a_s")
                        nc.vector.tensor_mul(
                            a_s, omask_rep[:, sl], invz_rep
                        )
                        ctx["a_s"] = a_s
                    nc.vector.tensor_mul(
                        outb[:, cb - 2, :], scr, ctx["a_s"]
                    )
                    nc.vector.tensor_add(
                        outb[:, cb - 2, :], outb[:, cb - 2, :],
                        mref[:, cb - 2, sl],
                    )
                if cb == 3:
                    oq = [nc.sync, nc.gpsimd]
                    for k in range(4):
                        oq[(s + k) % 2].dma_start(
                            out=out_r[k, :, sl], in_=outb[:, k, :]
                        )

            prev = None
            for s in range(NS):
                f8 = fbuf.tile([P, NB - 4, SLICE], F8, name="f8", tag="f8")
                fbf = fbuf.tile([P, 4, SLICE], BF16, name="fbf", tag="fbf")
                ctx = {"s": s, "f8": f8, "fbf": fbf}
                # interleave this slice's scores/exp chain with the previous
                # slice's apply matmuls in chunks of 9 per score group, so
                # the PE always has work while the exp chain serializes on
                # its single PSUM group buffer.
                for g in range(NG):
                    emit_scores_group(s, g, f8, fbf)
                    if prev is not None:
                        if g == 0:
                            prev["outb"] = obuf.tile(
                                [P, 4, SLICE], F32, name="outb", tag="outb"
                            )
                        if g % 2 == 0:
                            prev["pso"] = ps_ap.tile(
                                [P, SLICE], F32, name="psa", tag="psa"
                            )
                        for e in prev["mm"][9 * g : 9 * g + 9]:
                            emit_apply_mm(prev, e)
                        if g % 2 == 1:
                            emit_finalize_wave(prev, g // 2)
                    elif g < 7:
                        emit_warm_chunk()
                ctx["invz"] = emit_z(s, f8, fbf)
                ctx["mm"] = build_mm_list(s)
                prev = ctx
            # drain: last slice's apply + finalize
            prev["outb"] = obuf.tile([P, 4, SLICE], F32, name="outb", tag="outb")
            for cb in range(4):
                prev["pso"] = ps_ap.tile([P, SLICE], F32, name="psa", tag="psa")
                for e in prev["mm"][18 * cb : 18 * cb + 18]:
                    emit_apply_mm(prev, e)
                emit_finalize_wave(prev, cb)
            # keep the HAM clock at 8/8 while the last finalize + output
            # DMAs drain (the blend + stores run ~2x slower at half clock)
            for k in range(4):
                emit_warm_chunk()


def build():
    nc = bacc.Bacc(
        "TRN2",
        target_bir_lowering=False,
        debug=False,
        enable_asserts=False,
        num_devices=NCORES,
    )
    src = nc.dram_tensor("src", (C, HW), F32, kind="ExternalInput")
    ref = nc.dram_tensor("ref", (C, HW), F32, kind="ExternalInput")
    mask = nc.dram_tensor("mask", (HW,), F32, kind="ExternalInput")
    wT = nc.dram_tensor("wT", (C, 2 * CQ), BF16, kind="ExternalInput")
    out = nc.dram_tensor("out", (2 * C, HW), F32, kind="ExternalOutput")
    with tile.TileContext(nc) as tc:
        _build_body(tc, src, ref, mask, wT, out)
    nc.compile()
    return nc


_CACHE = {}


def _get_nc():
    if "nc" not in _CACHE:
        _CACHE["nc"] = build()
    return _CACHE["nc"]


def _in_maps(src_mask, src_feature, ref_feature, conv_w):
    import ml_dtypes

    n_batch = src_feature.shape[0]
    wT1 = np.asarray(conv_w, dtype=np.float32).T.astype(ml_dtypes.bfloat16)
    # duplicated columns: the conv then writes q into BOTH partition halves
    # of q2 in one matmul (the scores pairs need q at partitions 0-63 and
    # 64-127 for tile_position packing)
    wT = np.ascontiguousarray(np.concatenate([wT1, wT1], axis=1))
    maps = []
    for n in range(n_batch):
        maps.append(
            {
                "src": np.ascontiguousarray(
                    np.asarray(src_feature[n], dtype=np.float32).reshape(C, HW)
                ),
                "ref": np.ascontiguousarray(
                    np.asarray(ref_feature[n], dtype=np.float32).reshape(C, HW)
                ),
                "mask": np.ascontiguousarray(
                    np.asarray(src_mask[n], dtype=np.float32).reshape(HW)
                ),
                "wT": wT,
            }
        )
    return maps


def _install_ntff_hook():
    """The agent image's antenv lacks axon_hooks; recreate it so
    run_bass_kernel_spmd(trace=True) can capture NTFF profiles."""
    import sys
    import types

    if "antenv.axon_hooks" in sys.modules:
        return
    import antenv
    from trn_agent_boot.trn_boot import _ntff_profile_via_ctypes

    hook = _ntff_profile_via_ctypes("/opt/axon/libaxon_pjrt.so")
    mod = types.ModuleType("antenv.axon_hooks")
    mod._hook = hook
    mod.set_axon_ntff_profile_hook = lambda h: setattr(mod, "_hook", h)
    mod.get_axon_ntff_profile_hook = lambda: mod._hook
    sys.modules["antenv.axon_hooks"] = mod
    antenv.axon_hooks = mod


def run(src_mask, src_feature, ref_feature, conv_w, trace=False):
    """Run on 8 NeuronCores. Returns (output [N,2C,H,W], BassKernelResults)."""
    n_batch, c, h, w = src_feature.shape
    if trace:
        _install_ntff_hook()
    nc = _get_nc()
    maps = _in_maps(src_mask, src_feature, ref_feature, conv_w)
    res = bass_utils.run_bass_kernel_spmd(
        nc, maps, core_ids=list(range(NCORES)), trace=trace
    )
    out = np.stack([r["out"] for r in res.results], axis=0)
    return out.reshape(n_batch, 2 * c, h, w).astype(np.float32), res


def kernel(src_mask, src_feature, ref_feature, conv_w):
    out, _ = run(src_mask, src_feature, ref_feature, conv_w)
    return out
